# revision 13
# baseline (speedup 1.0000x reference)
"""MLA (multi-head latent attention) distributed Bass kernel for TRN2.

Full inputs in / full output out. Sharding: 8 cores = 2 batches x 4 head-groups
(4 heads each). Per-core pipeline, built around measured facts:
 - fp8 DoubleRow matmuls at 4 rotating 32-row tile_positions execute
   concurrently (~3.7x a full-array bf16 stream), so per-head QK (contraction
   64) runs as 4 parallel 32x128 tiles on fp8 q/k.
 - ScalarE exp at ~(N+352)/1.2ns is the pace-setter (~70us); the rest of the
   schedule hides under it.
 - fp8 GEMM noise does NOT average down (~5% per fp8 operand pair), so all
   projections run bf16 and only q/k are quantized to fp8 once, right before
   the row-tiled QK (error ~ the proven baseline's 0.8%).

Layouts: q/k packed [32h+di, par2, s] per head h with dim = 32*par2 + di so
QK runs DoubleRow per 32-partition head tile; scores^T [key, query] psum
quad-bank [128, 4head, 512]; one exp ACTIVATE per key-block round; causal diag
masked by a replicated upper-tri multiply on GpSimd; AV in two head-pair
passes (2 PSUM banks each, pass B of chunk c riding the front of chunk c+1)
with V carrying a 64-wide ones block so psum rows 64:128 hold softmax
denominators; normalize via reciprocal_approx_fast (exact-fit 2D tiles only —
custom DVE ops silently break on partition-offset/3D APs); W_o partials from
bf16 outT, evacuated by VectorE and DMA'd as bf16 [m, q]. Host sums 4
partials per batch and transposes.
"""

import math
import numpy as np
import ml_dtypes

import concourse.bass as bass
import concourse.bacc as bacc
import concourse.mybir as mybir
import concourse.tile as tile
from concourse import bass_utils

BF16 = ml_dtypes.bfloat16

D_MODEL = 1024
N_HEADS = 16
D_K = 64
D_C = 256
B, S = 2, 2048

NH = 4           # heads per core
CH = 512         # query chunk (psum bank width in fp32)
NCH = S // CH    # 4 chunks
P = 128
NKD = D_MODEL // P  # 8 k-blocks for x-side contraction
NKB = S // P     # 16 key blocks
ES_SLOTS = 20    # rotating exp-score tiles kept in SBUF
INV_SQRT_DK = 1.0 / math.sqrt(D_K)

_cached = None


def build_kernel():
    nc = bacc.Bacc("TRN2", debug=False, num_devices=8)
    dt = mybir.dt
    EXP = mybir.ActivationFunctionType.Exp
    DR = mybir.MatmulPerfMode.DoubleRow

    xT_d = nc.dram_tensor("xT", [P, NKD, S], dt.bfloat16, kind="ExternalInput")
    aq_d = nc.dram_tensor("aq", [P, NKD, 2, P], dt.bfloat16, kind="ExternalInput")
    wdkv_d = nc.dram_tensor("wdkv", [P, NKD, D_C], dt.bfloat16, kind="ExternalInput")
    wuk_d = nc.dram_tensor("wuk", [P, 2, 2, P], dt.bfloat16, kind="ExternalInput")
    wuv_d = nc.dram_tensor("wuv", [P, 2, D_C], dt.bfloat16, kind="ExternalInput")
    wo_d = nc.dram_tensor("wo", [2, P, D_MODEL], dt.bfloat16, kind="ExternalInput")
    tri4_d = nc.dram_tensor("tri4", [P, NH, P], dt.bfloat16, kind="ExternalInput")
    # output: y^T partials in [m, q] bf16 (summed in fp32 on host)
    yT_d = nc.dram_tensor("yT", [D_MODEL, S], dt.bfloat16, kind="ExternalOutput")

    with tile.TileContext(nc) as tc:
        with (
            tc.tile_pool(name="const", bufs=1) as const,
            tc.tile_pool(name="acts", bufs=1) as acts,
            tc.tile_pool(name="exps", bufs=1) as exps,
            tc.tile_pool(name="work", bufs=4) as work,
            tc.tile_pool(name="psqk", bufs=1, space="PSUM") as psqk_pool,
            tc.tile_pool(name="psav", bufs=1, space="PSUM") as psav_pool,
            tc.tile_pool(name="pp", bufs=2, space="PSUM") as pp_pool,
        ):
            # ---- loads, first-needed first ----
            wdkv = const.tile([P, NKD, D_C], dt.bfloat16, tag="wdkv")
            nc.sync.dma_start(wdkv[:], wdkv_d.ap())
            xT = const.tile([P, NKD, S], dt.bfloat16, tag="xT")
            nc.sync.dma_start(xT[:, :, 0:CH], xT_d.ap()[:, :, 0:CH])
            aq = const.tile([P, NKD, 2, P], dt.bfloat16, tag="aq")
            nc.sync.dma_start(aq[:], aq_d.ap())
            wuk = const.tile([P, 2, 2, P], dt.bfloat16, tag="wuk")
            nc.sync.dma_start(wuk[:], wuk_d.ap())
            wuv = const.tile([P, 2, D_C], dt.bfloat16, tag="wuv")
            nc.sync.dma_start(wuv[:], wuv_d.ap())
            tri4 = const.tile([P, NH, P], dt.bfloat16, tag="tri4")
            nc.sync.dma_start(tri4[:], tri4_d.ap())
            nc.sync.dma_start(xT[:, :, CH:S], xT_d.ap()[:, :, CH:S])
            wo = []
            for n in range(2):
                t = const.tile([P, D_MODEL], dt.bfloat16, name=f"wo{n}", tag=f"wo{n}")
                nc.sync.dma_start(t[:], wo_d.ap()[n])
                wo.append(t)

            # ---- persistent activations ----
            ckv = acts.tile([P, 2, S], dt.bfloat16, tag="ckv")
            qf8 = acts.tile([P, 2, S], dt.float8e4, tag="qf8")
            kf8 = acts.tile([P, 2, S], dt.float8e4, tag="kf8")
            v_sb = [acts.tile([P, NH, 2 * D_K], dt.bfloat16, name=f"v{kb}",
                              tag=f"v{kb}") for kb in range(NKB)]
            outT = [acts.tile([P, S], dt.bfloat16, name=f"outT{m}", tag=f"outT{m}")
                    for m in range(2)]
            es_sl = [exps.tile([P, NH, CH], dt.bfloat16, name=f"es{i}",
                               tag=f"es{i}") for i in range(ES_SLOTS)]

            for kb in range(NKB):
                nc.gpsimd.memset(v_sb[kb][:, :, D_K:2 * D_K], 1.0)

            psqk = psqk_pool.tile([P, NH, CH], dt.float32, tag="qk")
            psav = [psav_pool.tile([P, CH], dt.float32, name=f"av{i}",
                                   tag=f"av{i}") for i in range(2)]

            # ---- projection / output jobs (full-array mode fillers) ----
            def emit_ckv(ch, half):
                sl = slice(ch * CH, (ch + 1) * CH)
                pp = pp_pool.tile([P, CH], dt.float32, name="pp", tag="pp")
                for k in range(NKD):
                    nc.tensor.matmul(
                        pp[:], wdkv[:, k, half * P:(half + 1) * P],
                        xT[:, k, sl], start=(k == 0), stop=(k == NKD - 1))
                nc.vector.tensor_copy(ckv[:, half, sl], pp[:])

            def emit_q(ch, par2):
                sl = slice(ch * CH, (ch + 1) * CH)
                pp = pp_pool.tile([P, CH], dt.float32, name="pp", tag="pp")
                for k in range(NKD):
                    nc.tensor.matmul(
                        pp[:], aq[:, k, par2, :],
                        xT[:, k, sl], start=(k == 0), stop=(k == NKD - 1))
                nc.vector.tensor_copy(qf8[:, par2, sl], pp[:])

            def emit_k(ch, par2):
                sl = slice(ch * CH, (ch + 1) * CH)
                pp = pp_pool.tile([P, CH], dt.float32, name="pp", tag="pp")
                for half in range(2):
                    nc.tensor.matmul(pp[:], wuk[:, half, par2, :],
                                     ckv[:, half, sl],
                                     start=(half == 0), stop=(half == 1))
                nc.vector.tensor_copy(kf8[:, par2, sl], pp[:])

            def emit_v(kb):
                pp = pp_pool.tile([P, CH], dt.float32, name="pp", tag="pp")
                for half in range(2):
                    nc.tensor.matmul(pp[:, 0:D_C],
                                     ckv[:, half, kb * P:(kb + 1) * P],
                                     wuv[:, half, :],
                                     start=(half == 0), stop=(half == 1))
                nc.vector.tensor_copy(
                    v_sb[kb][:, :, 0:D_K],
                    pp[:, 0:D_C].rearrange("p (h d) -> p h d", h=NH))

            def emit_wo(ch, mb):
                # yT[m, q] partial = sum_d wo[d, m] outT[d, q]
                sl = slice(ch * CH, (ch + 1) * CH)
                pp = pp_pool.tile([P, CH], dt.float32, name="pp", tag="pp")
                for db in range(2):
                    nc.tensor.matmul(
                        pp[:], wo[db][:, mb * P:(mb + 1) * P],
                        outT[db][:, sl], start=(db == 0), stop=(db == 1))
                ysb = work.tile([P, CH], dt.bfloat16, name="ysb", tag="ysb")
                nc.vector.tensor_copy(ysb[:], pp[:])
                nc.sync.dma_start(yT_d.ap()[mb * P:(mb + 1) * P, sl], ysb[:])

            # ---- attention pieces ----
            def emit_qk(c, r, es_t):
                qoff = max(0, P * (r - 4 * c))
                w = CH - qoff
                qsl = slice(c * CH + qoff, (c + 1) * CH)
                for h in range(NH):
                    nc.tensor.matmul(
                        psqk[:, h, 0:w], kf8[32 * h:32 * (h + 1), :, r * P:(r + 1) * P],
                        qf8[32 * h:32 * (h + 1), :, qsl],
                        start=True, stop=True, perf_mode=DR,
                        tile_position=(32 * h, 0))
                nc.scalar.activation(es_t[:, :, 0:w], psqk[:, :, 0:w],
                                     EXP, scale=INV_SQRT_DK)
                if r >= 4 * c:
                    nc.gpsimd.tensor_mul(es_t[:, :, 0:P], es_t[:, :, 0:P], tri4[:])

            def emit_av(c, r, hpair, es_t, R):
                # hpair 0: heads 0,1 (pass A); hpair 1: heads 2,3 (pass B)
                qoff = max(0, P * (r - 4 * c))
                w = CH - qoff
                for e in range(2):
                    h = 2 * hpair + e
                    nc.tensor.matmul(
                        psav[e][:, qoff:CH], v_sb[r][:, h, :],
                        es_t[:, h, 0:w],
                        start=(r == 0), stop=(r == R - 1))

            def emit_norm(c, hpair):
                for e in range(2):
                    h = 2 * hpair + e
                    # custom DVE ops silently break on partition-offset or
                    # 3D-sliced APs: stage the denominator into an exact-fit
                    # [64, 512] tile with a standard copy (those remap
                    # offsets fine), run the approx reciprocal full-tile,
                    # then multiply.
                    dcp = work.tile([D_K, CH], dt.float32, name="dcp", tag="dcp")
                    rb = work.tile([D_K, CH], dt.float32, name="rb", tag="rb")
                    nc.vector.tensor_copy(dcp[:], psav[e][D_K:2 * D_K, :])
                    nc.vector.reciprocal_approx_fast(rb[:], dcp[:])
                    nc.vector.tensor_mul(
                        outT[h // 2][(h % 2) * D_K:(h % 2 + 1) * D_K,
                                     c * CH:(c + 1) * CH],
                        psav[e][0:D_K, :], rb[:])

            # ---- front: chunk 0 prep ----
            for half in range(2):
                emit_ckv(0, half)
            for par2 in range(2):
                emit_k(0, par2)
            for par2 in range(2):
                emit_q(0, par2)
            for kb in range(4):
                emit_v(kb)

            # ---- main loop ----
            # psav tiles are shared between pass A (heads 0,1) of chunk c and
            # pass B (heads 2,3) of chunk c-1, so all of pass B must be
            # emitted before chunk c's first pass-A matmul; pass A then
            # catches up on its key-block backlog at <=3 blocks per round.
            # wo(c-1) reads outT written by norm B of c-1, so those jobs also
            # live in the ordered `pre` list (emission order is binding: the
            # tile framework can only sequence against already-emitted
            # writes).
            g = 0  # global round counter -> es slot
            es_of = {}  # (c, r) -> es tile
            for c in range(NCH):
                R = 4 * (c + 1)
                pre = []
                if c > 0:
                    Rp = 4 * c
                    for r in range(Rp):
                        pre.append(
                            lambda cp=c - 1, rp=r, Rq=Rp:
                            emit_av(cp, rp, 1, es_of[(cp, rp)], Rq))
                    pre.append(lambda cp=c - 1: emit_norm(cp, 1))
                    for mb in range(D_MODEL // P):
                        pre.append(lambda ch=c - 1, m=mb: emit_wo(ch, m))
                fillers = []
                if c + 1 < NCH:
                    for half in range(2):
                        fillers.append(lambda ch=c + 1, hf=half: emit_ckv(ch, hf))
                    for par2 in range(2):
                        fillers.append(lambda ch=c + 1, p2=par2: emit_k(ch, p2))
                    for par2 in range(2):
                        fillers.append(lambda ch=c + 1, p2=par2: emit_q(ch, p2))
                    for kb in range(4 * c + 4, 4 * c + 8):
                        fillers.append(lambda kk=kb: emit_v(kk))

                done_a = 0  # pass-A key blocks completed
                for r in range(R):
                    es_t = es_sl[g % ES_SLOTS]
                    es_of[(c, r)] = es_t
                    g += 1
                    emit_qk(c, r, es_t)
                    if pre:
                        # drain pass B (+ norm + wo) of the previous chunk
                        # across the first ~3/4 of this chunk's rounds
                        npop = -(-len(pre) // max(1, 3 * R // 4 - r))
                        for fn in pre[:npop]:
                            fn()
                        pre = pre[npop:]
                    else:
                        for _ in range(min(3, r - done_a)):
                            emit_av(c, done_a, 0, es_of[(c, done_a)], R)
                            done_a += 1
                    if fillers:
                        npop = -(-len(fillers) // (R - r))
                        for fn in fillers[:npop]:
                            fn()
                        fillers = fillers[npop:]
                while done_a < R:
                    emit_av(c, done_a, 0, es_of[(c, done_a)], R)
                    done_a += 1
                emit_norm(c, 0)

            # ---- tail: pass B + wo of last chunk ----
            c = NCH - 1
            R = 4 * NCH
            for r in range(R):
                emit_av(c, r, 1, es_of[(c, r)], R)
            emit_norm(c, 1)
            for mb in range(D_MODEL // P):
                emit_wo(c, mb)

    nc.compile()
    return nc


def _fold(w):
    # [1024, M] -> [128, 8, M] partition-major
    m = w.shape[1]
    return np.ascontiguousarray(w.reshape(NKD, P, m).transpose(1, 0, 2))


# q/k column packing: packed col (par2, h, di) <- original col 64h + 32par2 + di
_PERMQ = np.array([64 * h + 32 * par2 + di
                   for par2 in range(2) for h in range(NH) for di in range(32)])


def _prep_inputs(x, W_dq, W_uq, W_dkv, W_uk, W_uv, W_o):
    x = np.asarray(x, np.float32)
    W_dq = np.asarray(W_dq, np.float32)
    W_uq = np.asarray(W_uq, np.float32)
    W_dkv = np.asarray(W_dkv, np.float32)
    W_uk = np.asarray(W_uk, np.float32)
    W_uv = np.asarray(W_uv, np.float32)
    W_o = np.asarray(W_o, np.float32)

    tri = np.triu(np.ones((P, P), dtype=np.float32))  # valid where q >= k
    tri4 = np.ascontiguousarray(
        np.broadcast_to(tri[:, None, :], (P, NH, P))).astype(BF16)

    wdkv = _fold(W_dkv).astype(BF16)
    in_maps = []
    for core in range(8):
        b, hg = divmod(core, 4)
        cs = slice(hg * NH * D_K, (hg + 1) * NH * D_K)
        xT = np.ascontiguousarray(x[b].T)                      # [1024, 2048]
        aq = (W_dq @ W_uq[:, cs])[:, _PERMQ]                   # [1024, 256]
        wuk = W_uk[:, cs][:, _PERMQ]                           # [256, 256]
        wuk4 = wuk.reshape(2, P, 2, P).transpose(1, 0, 2, 3)   # [ki, chalf, par2, m]
        wuv = W_uv[:, cs].reshape(2, P, D_C).transpose(1, 0, 2)
        in_maps.append({
            "xT": _fold(xT).astype(BF16),
            "aq": _fold(aq).reshape(P, NKD, 2, P).astype(BF16),
            "wdkv": wdkv,
            "wuk": np.ascontiguousarray(wuk4).astype(BF16),
            "wuv": np.ascontiguousarray(wuv).astype(BF16),
            "wo": W_o[cs, :].astype(BF16).reshape(2, P, D_MODEL),
            "tri4": tri4,
        })
    return in_maps


def run(inputs, trace=False, **kw):
    global _cached
    if _cached is None:
        _cached = build_kernel()
    in_maps = _prep_inputs(**inputs)
    res = bass_utils.run_bass_kernel_spmd(
        _cached, in_maps, core_ids=list(range(8)), trace=trace, **kw)
    ys = [res.results[c]["yT"].astype(np.float32) for c in range(8)]
    out = np.stack([
        (ys[0] + ys[1] + ys[2] + ys[3]).T,
        (ys[4] + ys[5] + ys[6] + ys[7]).T,
    ]).astype(np.float32)
    return out, res


def kernel(**inputs):
    out, _ = run(inputs)
    return out


# revision 18
# speedup vs baseline: 1.0043x; 1.0043x over previous
"""MLA (multi-head latent attention) distributed Bass kernel for TRN2.

Full inputs in / full output out. Sharding: 8 cores = 2 batches x 4 head-groups
(4 heads each). Per-core pipeline, built around measured facts:
 - fp8 DoubleRow matmuls at 4 rotating 32-row tile_positions execute
   concurrently (~3.7x a full-array bf16 stream), so per-head QK (contraction
   64) runs as 4 parallel 32x128 tiles on fp8 q/k.
 - ScalarE exp at ~(N+352)/1.2ns is the pace-setter (~70us); the rest of the
   schedule hides under it.
 - fp8 GEMM noise does NOT average down (~5% per fp8 operand pair), so all
   projections run bf16 and only q/k are quantized to fp8 once, right before
   the row-tiled QK (error ~ the proven baseline's 0.8%).

Layouts: q/k packed [32h+di, par2, s] per head h with dim = 32*par2 + di so
QK runs DoubleRow per 32-partition head tile; scores^T [key, query] psum
quad-bank [128, 4head, 512]; one exp ACTIVATE per key-block round; causal diag
masked by a replicated upper-tri multiply on GpSimd; AV in two head-pair
passes (2 PSUM banks each, pass B of chunk c riding the front of chunk c+1)
with V carrying a 64-wide ones block so psum rows 64:128 hold softmax
denominators; normalize via reciprocal_approx_fast (exact-fit 2D tiles only —
custom DVE ops silently break on partition-offset/3D APs); W_o partials from
bf16 outT, evacuated by VectorE and DMA'd as bf16 [m, q]. Host sums 4
partials per batch and transposes.
"""

import math
import numpy as np
import ml_dtypes

import concourse.bass as bass
import concourse.bacc as bacc
import concourse.mybir as mybir
import concourse.tile as tile
from concourse import bass_utils

BF16 = ml_dtypes.bfloat16

D_MODEL = 1024
N_HEADS = 16
D_K = 64
D_C = 256
B, S = 2, 2048

NH = 4           # heads per core
CH = 512         # query chunk (psum bank width in fp32)
NCH = S // CH    # 4 chunks
P = 128
NKD = D_MODEL // P  # 8 k-blocks for x-side contraction
NKB = S // P     # 16 key blocks
ES_SLOTS = 20    # rotating exp-score tiles kept in SBUF
INV_SQRT_DK = 1.0 / math.sqrt(D_K)

_cached = None


def build_kernel():
    nc = bacc.Bacc("TRN2", debug=False, num_devices=8)
    dt = mybir.dt
    EXP = mybir.ActivationFunctionType.Exp
    DR = mybir.MatmulPerfMode.DoubleRow

    xT_d = nc.dram_tensor("xT", [P, NKD, S], dt.bfloat16, kind="ExternalInput")
    aq_d = nc.dram_tensor("aq", [P, NKD, 2, P], dt.bfloat16, kind="ExternalInput")
    wdkv_d = nc.dram_tensor("wdkv", [P, NKD, D_C], dt.bfloat16, kind="ExternalInput")
    wuk_d = nc.dram_tensor("wuk", [P, 2, 2, P], dt.bfloat16, kind="ExternalInput")
    wuv_d = nc.dram_tensor("wuv", [P, 2, D_C], dt.bfloat16, kind="ExternalInput")
    wo_d = nc.dram_tensor("wo", [2, P, D_MODEL], dt.bfloat16, kind="ExternalInput")
    tri4_d = nc.dram_tensor("tri4", [P, NH, P], dt.bfloat16, kind="ExternalInput")
    # output: y^T partials in [m, q] bf16 (summed in fp32 on host)
    yT_d = nc.dram_tensor("yT", [D_MODEL, S], dt.bfloat16, kind="ExternalOutput")

    with tile.TileContext(nc) as tc:
        with (
            tc.tile_pool(name="const", bufs=1) as const,
            tc.tile_pool(name="acts", bufs=1) as acts,
            tc.tile_pool(name="exps", bufs=1) as exps,
            tc.tile_pool(name="work", bufs=4) as work,
            tc.tile_pool(name="psqk", bufs=1, space="PSUM") as psqk_pool,
            tc.tile_pool(name="psav", bufs=1, space="PSUM") as psav_pool,
            tc.tile_pool(name="pp", bufs=2, space="PSUM") as pp_pool,
        ):
            # ---- loads, first-needed first ----
            aq = const.tile([P, NKD, 2, P], dt.bfloat16, tag="aq")
            nc.sync.dma_start(aq[:], aq_d.ap())
            xT = const.tile([P, NKD, S], dt.bfloat16, tag="xT")
            nc.sync.dma_start(xT[:, :, 0:CH], xT_d.ap()[:, :, 0:CH])
            wdkv = const.tile([P, NKD, D_C], dt.bfloat16, tag="wdkv")
            nc.sync.dma_start(wdkv[:], wdkv_d.ap())
            wuk = const.tile([P, 2, 2, P], dt.bfloat16, tag="wuk")
            nc.sync.dma_start(wuk[:], wuk_d.ap())
            wuv = const.tile([P, 2, D_C], dt.bfloat16, tag="wuv")
            nc.sync.dma_start(wuv[:], wuv_d.ap())
            tri4 = const.tile([P, NH, P], dt.bfloat16, tag="tri4")
            nc.sync.dma_start(tri4[:], tri4_d.ap())
            nc.sync.dma_start(xT[:, :, CH:S], xT_d.ap()[:, :, CH:S])
            wo = []
            for n in range(2):
                t = const.tile([P, D_MODEL], dt.bfloat16, name=f"wo{n}", tag=f"wo{n}")
                nc.sync.dma_start(t[:], wo_d.ap()[n])
                wo.append(t)
            # warm the exp table set (~2.7us ACT_TABLE_LOAD) under the DMAs
            warm = work.tile([1, 2], dt.float32, name="warm", tag="warm")
            nc.gpsimd.memset(warm[:], 0.0)
            nc.scalar.activation(warm[:, 1:2], warm[:, 0:1],
                                 EXP, scale=1.0)

            # ---- persistent activations ----
            ckv = acts.tile([P, 2, S], dt.bfloat16, tag="ckv")
            qf8 = acts.tile([P, 2, S], dt.float8e4, tag="qf8")
            kf8 = acts.tile([P, 2, S], dt.float8e4, tag="kf8")
            v_sb = [acts.tile([P, NH, 2 * D_K], dt.bfloat16, name=f"v{kb}",
                              tag=f"v{kb}") for kb in range(NKB)]
            outT = [acts.tile([P, S], dt.bfloat16, name=f"outT{m}", tag=f"outT{m}")
                    for m in range(2)]
            es_sl = [exps.tile([P, NH, CH], dt.bfloat16, name=f"es{i}",
                               tag=f"es{i}") for i in range(ES_SLOTS)]

            for kb in range(NKB):
                nc.gpsimd.memset(v_sb[kb][:, :, D_K:2 * D_K], 1.0)

            psqk = psqk_pool.tile([P, NH, CH], dt.float32, tag="qk")
            psav = [psav_pool.tile([P, CH], dt.float32, name=f"av{i}",
                                   tag=f"av{i}") for i in range(2)]

            # ---- projection / output jobs (full-array mode fillers) ----
            def emit_ckv(ch, half):
                sl = slice(ch * CH, (ch + 1) * CH)
                pp = pp_pool.tile([P, CH], dt.float32, name="pp", tag="pp")
                for k in range(NKD):
                    nc.tensor.matmul(
                        pp[:], wdkv[:, k, half * P:(half + 1) * P],
                        xT[:, k, sl], start=(k == 0), stop=(k == NKD - 1))
                nc.vector.tensor_copy(ckv[:, half, sl], pp[:])

            def emit_q(ch, par2):
                sl = slice(ch * CH, (ch + 1) * CH)
                pp = pp_pool.tile([P, CH], dt.float32, name="pp", tag="pp")
                for k in range(NKD):
                    nc.tensor.matmul(
                        pp[:], aq[:, k, par2, :],
                        xT[:, k, sl], start=(k == 0), stop=(k == NKD - 1))
                nc.vector.tensor_copy(qf8[:, par2, sl], pp[:])

            def emit_k(ch, par2):
                sl = slice(ch * CH, (ch + 1) * CH)
                pp = pp_pool.tile([P, CH], dt.float32, name="pp", tag="pp")
                for half in range(2):
                    nc.tensor.matmul(pp[:], wuk[:, half, par2, :],
                                     ckv[:, half, sl],
                                     start=(half == 0), stop=(half == 1))
                nc.vector.tensor_copy(kf8[:, par2, sl], pp[:])

            def emit_v(kb):
                pp = pp_pool.tile([P, CH], dt.float32, name="pp", tag="pp")
                for half in range(2):
                    nc.tensor.matmul(pp[:, 0:D_C],
                                     ckv[:, half, kb * P:(kb + 1) * P],
                                     wuv[:, half, :],
                                     start=(half == 0), stop=(half == 1))
                nc.vector.tensor_copy(
                    v_sb[kb][:, :, 0:D_K],
                    pp[:, 0:D_C].rearrange("p (h d) -> p h d", h=NH))

            def emit_wo(ch, mb):
                # yT[m, q] partial = sum_d wo[d, m] outT[d, q]
                sl = slice(ch * CH, (ch + 1) * CH)
                pp = pp_pool.tile([P, CH], dt.float32, name="pp", tag="pp")
                for db in range(2):
                    nc.tensor.matmul(
                        pp[:], wo[db][:, mb * P:(mb + 1) * P],
                        outT[db][:, sl], start=(db == 0), stop=(db == 1))
                ysb = work.tile([P, CH], dt.bfloat16, name="ysb", tag="ysb")
                nc.vector.tensor_copy(ysb[:], pp[:])
                nc.sync.dma_start(yT_d.ap()[mb * P:(mb + 1) * P, sl], ysb[:])

            # ---- attention pieces ----
            def emit_qk(c, r, es_t):
                qoff = max(0, P * (r - 4 * c))
                w = CH - qoff
                qsl = slice(c * CH + qoff, (c + 1) * CH)
                for h in range(NH):
                    nc.tensor.matmul(
                        psqk[:, h, 0:w], kf8[32 * h:32 * (h + 1), :, r * P:(r + 1) * P],
                        qf8[32 * h:32 * (h + 1), :, qsl],
                        start=True, stop=True, perf_mode=DR,
                        tile_position=(32 * h, 0))
                nc.scalar.activation(es_t[:, :, 0:w], psqk[:, :, 0:w],
                                     EXP, scale=INV_SQRT_DK)
                if r >= 4 * c:
                    nc.gpsimd.tensor_mul(es_t[:, :, 0:P], es_t[:, :, 0:P], tri4[:])

            def emit_av(c, r, hpair, es_t, R, pair=None):
                # hpair 0: heads 0,1 (pass A); hpair 1: heads 2,3 (pass B)
                pair = pair if pair is not None else psav
                qoff = max(0, P * (r - 4 * c))
                w = CH - qoff
                for e in range(2):
                    h = 2 * hpair + e
                    nc.tensor.matmul(
                        pair[e][:, qoff:CH], v_sb[r][:, h, :],
                        es_t[:, h, 0:w],
                        start=(r == 0), stop=(r == R - 1))

            def emit_norm(c, hpair, pair=None):
                pair = pair if pair is not None else psav
                for e in range(2):
                    h = 2 * hpair + e
                    # custom DVE ops silently break on partition-offset or
                    # 3D-sliced APs: stage the denominator into an exact-fit
                    # [64, 512] tile with a standard copy (those remap
                    # offsets fine), run the approx reciprocal full-tile,
                    # then multiply.
                    dcp = work.tile([D_K, CH], dt.float32, name="dcp", tag="dcp")
                    rb = work.tile([D_K, CH], dt.float32, name="rb", tag="rb")
                    nc.vector.tensor_copy(dcp[:], pair[e][D_K:2 * D_K, :])
                    nc.vector.reciprocal_approx_fast(rb[:], dcp[:])
                    nc.vector.tensor_mul(
                        outT[h // 2][(h % 2) * D_K:(h % 2 + 1) * D_K,
                                     c * CH:(c + 1) * CH],
                        pair[e][0:D_K, :], rb[:])

            # ---- front: chunk 0 prep (v jobs ride the round-0 fillers) ----
            for par2 in range(2):
                emit_q(0, par2)
            for half in range(2):
                emit_ckv(0, half)
            for par2 in range(2):
                emit_k(0, par2)

            # ---- main loop ----
            # psav tiles are shared between pass A (heads 0,1) of chunk c and
            # pass B (heads 2,3) of chunk c-1, so all of pass B must be
            # emitted before chunk c's first pass-A matmul; pass A then
            # catches up on its key-block backlog at <=3 blocks per round.
            # wo(c-1) reads outT written by norm B of c-1, so those jobs also
            # live in the ordered `pre` list (emission order is binding: the
            # tile framework can only sequence against already-emitted
            # writes).
            g = 0  # global round counter -> es slot
            es_of = {}  # (c, r) -> es tile
            ppb = [None, None]  # last-chunk pass-B accumulators (pp banks)
            for c in range(NCH):
                R = 4 * (c + 1)
                last = c == NCH - 1
                pre = []
                if c > 0:
                    Rp = 4 * c
                    for r in range(Rp):
                        pre.append(
                            lambda cp=c - 1, rp=r, Rq=Rp:
                            emit_av(cp, rp, 1, es_of[(cp, rp)], Rq))
                    pre.append(lambda cp=c - 1: emit_norm(cp, 1))
                    for mb in range(D_MODEL // P):
                        pre.append(lambda ch=c - 1, m=mb: emit_wo(ch, m))
                fillers = []
                if c == 0:
                    for kb in range(4):
                        fillers.append(lambda kk=kb: emit_v(kk))
                if c + 1 < NCH:
                    for half in range(2):
                        fillers.append(lambda ch=c + 1, hf=half: emit_ckv(ch, hf))
                    for par2 in range(2):
                        fillers.append(lambda ch=c + 1, p2=par2: emit_k(ch, p2))
                    for par2 in range(2):
                        fillers.append(lambda ch=c + 1, p2=par2: emit_q(ch, p2))
                    for kb in range(4 * c + 4, 4 * c + 8):
                        fillers.append(lambda kk=kb: emit_v(kk))

                done_a = 0  # pass-A key blocks completed
                done_b = 0  # (last chunk) pass-B key blocks completed
                for r in range(R):
                    es_t = es_sl[g % ES_SLOTS]
                    es_of[(c, r)] = es_t
                    g += 1
                    emit_qk(c, r, es_t)
                    if pre:
                        # drain pass B (+ norm + wo) of the previous chunk
                        # across the first half of this chunk's rounds
                        npop = -(-len(pre) // max(1, R // 2 - r))
                        for fn in pre[:npop]:
                            fn()
                        pre = pre[npop:]
                    else:
                        for _ in range(min(3, r - done_a)):
                            emit_av(c, done_a, 0, es_of[(c, done_a)], R)
                            done_a += 1
                        if last:
                            # pass B rides the freed pp banks in-round
                            if ppb[0] is None:
                                ppb = [pp_pool.tile([P, CH], dt.float32,
                                                    name="ppb", tag="pp")
                                       for _ in range(2)]
                            for _ in range(min(2, r - done_b)):
                                emit_av(c, done_b, 1, es_of[(c, done_b)], R,
                                        pair=ppb)
                                done_b += 1
                    if fillers:
                        npop = -(-len(fillers) // (R - r))
                        for fn in fillers[:npop]:
                            fn()
                        fillers = fillers[npop:]
                while done_a < R:
                    emit_av(c, done_a, 0, es_of[(c, done_a)], R)
                    done_a += 1
                emit_norm(c, 0)

            # ---- tail: finish pass B + wo of last chunk ----
            c = NCH - 1
            R = 4 * NCH
            while done_b < R:
                emit_av(c, done_b, 1, es_of[(c, done_b)], R, pair=ppb)
                done_b += 1
            emit_norm(c, 1, pair=ppb)
            for mb in range(D_MODEL // P):
                emit_wo(c, mb)

    nc.compile()
    return nc


def _fold(w):
    # [1024, M] -> [128, 8, M] partition-major
    m = w.shape[1]
    return np.ascontiguousarray(w.reshape(NKD, P, m).transpose(1, 0, 2))


# q/k column packing: packed col (par2, h, di) <- original col 64h + 32par2 + di
_PERMQ = np.array([64 * h + 32 * par2 + di
                   for par2 in range(2) for h in range(NH) for di in range(32)])


def _prep_inputs(x, W_dq, W_uq, W_dkv, W_uk, W_uv, W_o):
    x = np.asarray(x, np.float32)
    W_dq = np.asarray(W_dq, np.float32)
    W_uq = np.asarray(W_uq, np.float32)
    W_dkv = np.asarray(W_dkv, np.float32)
    W_uk = np.asarray(W_uk, np.float32)
    W_uv = np.asarray(W_uv, np.float32)
    W_o = np.asarray(W_o, np.float32)

    tri = np.triu(np.ones((P, P), dtype=np.float32))  # valid where q >= k
    tri4 = np.ascontiguousarray(
        np.broadcast_to(tri[:, None, :], (P, NH, P))).astype(BF16)

    wdkv = _fold(W_dkv).astype(BF16)
    in_maps = []
    for core in range(8):
        b, hg = divmod(core, 4)
        cs = slice(hg * NH * D_K, (hg + 1) * NH * D_K)
        xT = np.ascontiguousarray(x[b].T)                      # [1024, 2048]
        aq = (W_dq @ W_uq[:, cs])[:, _PERMQ]                   # [1024, 256]
        wuk = W_uk[:, cs][:, _PERMQ]                           # [256, 256]
        wuk4 = wuk.reshape(2, P, 2, P).transpose(1, 0, 2, 3)   # [ki, chalf, par2, m]
        wuv = W_uv[:, cs].reshape(2, P, D_C).transpose(1, 0, 2)
        in_maps.append({
            "xT": _fold(xT).astype(BF16),
            "aq": _fold(aq).reshape(P, NKD, 2, P).astype(BF16),
            "wdkv": wdkv,
            "wuk": np.ascontiguousarray(wuk4).astype(BF16),
            "wuv": np.ascontiguousarray(wuv).astype(BF16),
            "wo": W_o[cs, :].astype(BF16).reshape(2, P, D_MODEL),
            "tri4": tri4,
        })
    return in_maps


def run(inputs, trace=False, **kw):
    global _cached
    if _cached is None:
        _cached = build_kernel()
    in_maps = _prep_inputs(**inputs)
    res = bass_utils.run_bass_kernel_spmd(
        _cached, in_maps, core_ids=list(range(8)), trace=trace, **kw)
    ys = [res.results[c]["yT"].astype(np.float32) for c in range(8)]
    out = np.stack([
        (ys[0] + ys[1] + ys[2] + ys[3]).T,
        (ys[4] + ys[5] + ys[6] + ys[7]).T,
    ]).astype(np.float32)
    return out, res


def kernel(**inputs):
    out, _ = run(inputs)
    return out
